# revision 4
# baseline (speedup 1.0000x reference)
"""CstLoss on Trainium2 — self-contained Bass/Tile SPMD kernel (8 NeuronCores).

Reference math (per [N=64, C=17, H=128, W=128] f32 pair output/target):
  h/w marginal means of each map -> softmax over the 128-axis -> l2
  normalize -> sim_pos = mean of matched-channel cosines, sim = sum of
  mean-over-batch all-pairs cosines, loss = -log(sim_pos/sim)/C/N.

Key algebra:
  * softmax denominator cancels under l2 normalization, and exp's argument
    S/W is O(1) here, so no max-subtraction is needed: each projection only
    needs e = exp(S/W) and ssq = sum(e^2) = sum exp(2S/W) (second Exp pass,
    no Square/Sqrt tables; ssq ships as partial accumulator columns that
    the host adds).
  * The device ships e (fp16) and ssq partials (f32); the host (f64)
    finishes: q = e/sqrt(ssq), matched dots, channel sums, the two scalar
    all-reduces, and the log.

Device pipeline (memory-bound; inputs stream once at HBM line rate):
  * Inputs are cast f32->fp16 during the load (SWDGE gpsimd DMA); PE and
    DVE only ever see fp16 operands. 7 chunks per tensor (6x20 + 8 h-rows)
    keep the DVE reduce and the exp tail tracking the DMA stream closely.
  * Main 128 maps sit one-map-per-partition. h-projection: DVE segmented
    reduce per chunk. w-projection: per h-row matmul with the data slice as
    the STATIONARY operand and an fp16 identity as the moving operand --
    a transpose through the regular matmul path, so PSUM accumulates in
    f32, FWL hides the weight loads, and HAM warms (unlike is_transpose).
  * 8-map-per-tensor tail lives in h-on-partition layout; its h-projection
    is a DVE reduce, its w-projection is 4 accumulating matmuls against
    one-hot column blocks; ssq over partitions via a ones-vector matmul.
    Results ship in device layout; the host reindexes.
  * Tensor o's finalize (wt copy, back-transpose, exps, stores on the sync
    HWDGE ring) overlaps tensor t's last loads; t's stores ride the scalar
    HWDGE ring so the two store chains don't serialize.
"""

import contextlib
import ctypes
import sys
import types
from contextlib import ExitStack

import numpy as np

import concourse.bacc as bacc
import concourse.tile as tile
from concourse import mybir
from concourse.bass_utils import run_bass_kernel_spmd

F32 = mybir.dt.float32
F16 = mybir.dt.float16
AX = mybir.AxisListType
ACT = mybir.ActivationFunctionType

N, C, H, W = 64, 17, 128, 128
NCORES = 8
NLOC = N // NCORES           # 8 batch entries per core
MAPS = NLOC * C              # 136 maps per tensor per core
MAIN = 128                   # maps in the main batch
TAIL = MAPS - MAIN           # 8 maps in the tail
CHUNKS = (20, 20, 20, 20, 20, 20, 8)   # h-rows per main chunk


def _install_ntff_hook():
    """Provide antenv.axon_hooks if the image lacks it (needed only when
    run_bass_kernel_spmd is called with trace=True; harmless otherwise)."""
    if "antenv.axon_hooks" in sys.modules:
        return
    so_path = "/opt/axon/libaxon_pjrt.so"
    hook = None
    try:
        lib = ctypes.CDLL(so_path)
        if hasattr(lib, "axon_start_nrt_profile"):
            lib.axon_start_nrt_profile.argtypes = [
                ctypes.POINTER(ctypes.c_int64),
                ctypes.c_size_t,
            ]
            lib.axon_start_nrt_profile.restype = ctypes.c_int64
            lib.axon_stop_nrt_profile.argtypes = [ctypes.c_char_p]
            lib.axon_stop_nrt_profile.restype = ctypes.c_int64

            @contextlib.contextmanager
            def _hook(output_dir, device_ids):
                import jax

                jax.devices()
                if device_ids:
                    ids = (ctypes.c_int64 * len(device_ids))(*device_ids)
                    rc = lib.axon_start_nrt_profile(ids, len(device_ids))
                else:
                    rc = lib.axon_start_nrt_profile(None, 0)
                if rc != 0:
                    raise RuntimeError(f"axon_start_nrt_profile rc={rc}")
                try:
                    yield
                finally:
                    n = lib.axon_stop_nrt_profile(str(output_dir).encode())
                    print(f"profile: {n} file(s) in {output_dir}", file=sys.stderr)

            hook = _hook
    except OSError:
        pass
    mod = types.ModuleType("antenv.axon_hooks")
    mod.get_axon_ntff_profile_hook = lambda: hook
    mod.set_axon_ntff_profile_hook = lambda h: None
    sys.modules["antenv.axon_hooks"] = mod


_install_ntff_hook()


def _body(tc, o_d, t_d, id_d, on_d, ek_d, eo_d, so_d, et_d, st_d,
          eth_d, sth_d, etw_d, stw_d):
    nc = tc.nc
    with ExitStack() as ctx:
        consts = ctx.enter_context(tc.tile_pool(name="consts", bufs=1))
        chunks = ctx.enter_context(tc.tile_pool(name="chunks", bufs=8))
        tailp = ctx.enter_context(tc.tile_pool(name="tailp", bufs=1))
        projp = ctx.enter_context(tc.tile_pool(name="projp", bufs=1))
        workp = ctx.enter_context(tc.tile_pool(name="workp", bufs=2))
        outp = ctx.enter_context(tc.tile_pool(name="outp", bufs=1))
        # PSUM: distinct tiles only, no slot rotation (slot reuse with
        # concurrent PE traffic wedges the device: NRT status 101).
        accps = ctx.enter_context(tc.tile_pool(name="accps", bufs=1, space="PSUM"))

        ident = consts.tile([128, 128], F16)
        nc.sync.dma_start(ident[:], id_d)
        ones = consts.tile([128, 1], F16)
        nc.sync.dma_start(ones[:], on_d)
        ek = consts.tile([128, 4 * 4], F16)
        nc.sync.dma_start(ek[:], ek_d)

        proj_o = projp.tile([128, W], F32)
        proj_t = projp.tile([128, W], F32)
        wt_o = accps.tile([128, 128], F32)
        wt_t = accps.tile([128, 128], F32)
        wb_o = accps.tile([128, 128], F32)
        wb_t = accps.tile([128, 128], F32)
        tlm = accps.tile([4, 512], F32, name="tlm")
        sth_ps = accps.tile([1, 16], F32, name="sthps")

        e_o = outp.tile([128, 2 * W], F16)
        e_t = outp.tile([128, 2 * W], F16)
        # ssq partial columns: 0 = h-rows 0:120, 1 = h-rows 120:128, 2 = w,
        # 3 = unused pad; the host adds cols 0+1.
        ssq_o = outp.tile([128, 4], F32)
        ssq_t = outp.tile([128, 4], F32)

        nchunks = len(CHUNKS)
        starts = [sum(CHUNKS[:i]) for i in range(nchunks)]
        SPLIT = starts[nchunks - 1]      # 120: ssq partial boundary

        tail2d = tailp.tile([128, 2 * TAIL * W], F16)
        tv = tail2d.rearrange("p (m w) -> p m w", w=W)

        def load_chunk(ti, x_d, c):
            r0, rows = starts[c], CHUNKS[c]
            chunk = chunks.tile([128, rows * W], F16, tag="chunk",
                                name=f"chunk{ti}_{c}")
            nc.gpsimd.dma_start(chunk[:], x_d[0:MAIN, r0:r0 + rows, :])
            return chunk

        def process_chunk(c, chunk, proj, e, wt):
            r0, rows = starts[c], CHUNKS[c]
            cv = chunk.rearrange("p (h w) -> p h w", w=W)
            nc.vector.reduce_sum(proj[:, r0:r0 + rows], cv, axis=AX.X)
            nc.scalar.activation(e[:, r0:r0 + rows], proj[:, r0:r0 + rows],
                                 ACT.Exp, scale=1.0 / W)
            for j in range(rows):
                # out[w, map] += chunk[map, j, w]: data slice is the
                # stationary operand, identity streams -> f32 PSUM accum.
                nc.tensor.matmul(
                    wt[:], cv[:, j, :], ident[:],
                    start=(c == 0 and j == 0),
                    stop=(c == nchunks - 1 and j == rows - 1),
                )

        def finalize(ti, proj, e, ssq, wt, wb, e_d, s_d, dma):
            # last ssq-h partial + w-projection back-transpose + exps + ship
            dmpb = workp.tile([128, W - SPLIT], F16, tag="dmpb", name=f"dmpb{ti}")
            nc.scalar.activation(dmpb[:], proj[:, SPLIT:W], ACT.Exp,
                                 scale=2.0 / W, accum_out=ssq[:, 1:2])
            wts = workp.tile([128, 128], F16, tag="wts", name=f"wts{ti}")
            nc.vector.tensor_copy(wts[:], wt[:])
            nc.tensor.matmul(wb[:], wts[:], ident[:], skip_group_check=True)
            nc.scalar.activation(e[:, W:2 * W], wb[:], ACT.Exp, scale=1.0 / W)
            dmpw = workp.tile([128, W], F16, tag="dmpw", name=f"dmpw{ti}")
            nc.scalar.activation(dmpw[:], wb[:], ACT.Exp, scale=2.0 / W,
                                 accum_out=ssq[:, 2:3])
            dma(e_d, e[:])
            dma(s_d, ssq[:])

        tensors = ((0, o_d, proj_o, wt_o, wb_o, e_o, ssq_o, eo_d, so_d),
                   (1, t_d, proj_t, wt_t, wb_t, e_t, ssq_t, et_d, st_d))

        # ---- queue all loads on the SWDGE ring: first chunks, then the
        # tail (lands ~30us in, compute hides mid-stream), then the rest.
        loaded = {}
        for c in range(nchunks):
            for ti, x_d, *_ in tensors:
                loaded[(ti, c)] = load_chunk(ti, x_d, c)
            if c == 1:
                nc.gpsimd.dma_start(tv[:, 0:TAIL, :],
                                    o_d[MAIN:MAPS].rearrange("m h w -> h m w"))
                nc.gpsimd.dma_start(tv[:, TAIL:2 * TAIL, :],
                                    t_d[MAIN:MAPS].rearrange("m h w -> h m w"))

        for c in range(nchunks):
            for ti, x_d, proj, wt, wb, e, ssq, e_d, s_d in tensors:
                if c == nchunks - 1 and ti == 1:
                    # o's finalize overlaps t's last chunk
                    finalize(0, proj_o, e_o, ssq_o, wt_o, wb_o, eo_d, so_d,
                             nc.sync.dma_start)
                process_chunk(c, loaded[(ti, c)], proj, e, wt)
                if c == nchunks - 2:
                    # h-rows 0:120 reduced: early bulk ssq-h partial
                    dmpa = workp.tile([128, SPLIT], F16, tag="dmpa",
                                      name=f"dmpa{ti}")
                    nc.scalar.activation(dmpa[:], proj[:, 0:SPLIT], ACT.Exp,
                                         scale=2.0 / W, accum_out=ssq[:, 0:1])
            if c == 3:
                # ---- tail compute (data landed ~30us) ----
                R = tailp.tile([128, 2 * TAIL], F32)
                nc.vector.reduce_sum(R[:], tv, axis=AX.X)
                eth = tailp.tile([128, 2 * TAIL], F16)
                nc.scalar.activation(eth[:], R[:], ACT.Exp, scale=1.0 / W)
                nc.sync.dma_start(eth_d, eth[:])
                esqh = tailp.tile([128, 2 * TAIL], F16)
                nc.scalar.activation(esqh[:], R[:], ACT.Exp, scale=2.0 / W)
                nc.tensor.matmul(sth_ps[:], ones[:], esqh[:],
                                 skip_group_check=True)
                sth = tailp.tile([1, 16], F32)
                nc.vector.tensor_copy(sth[:], sth_ps[:])
                nc.sync.dma_start(sth_d, sth[:])
                for k in range(4):
                    nc.tensor.matmul(
                        tlm[:], ek[:, 4 * k:4 * k + 4],
                        tail2d[:, k * 512:(k + 1) * 512],
                        start=(k == 0), stop=(k == 3),
                        skip_group_check=True,
                    )
                etw = tailp.tile([4, 512], F16)
                nc.scalar.activation(etw[:], tlm[:], ACT.Exp, scale=1.0 / W)
                nc.sync.dma_start(etw_d, etw[:])
                esqw = tailp.tile([4, 512], F16)
                nc.scalar.activation(esqw[:], tlm[:], ACT.Exp, scale=2.0 / W)
                stw = tailp.tile([4, 4], F32)
                nc.vector.reduce_sum(
                    stw[:], esqw.rearrange("p (m w) -> p m w", w=W), axis=AX.X)
                nc.sync.dma_start(stw_d, stw[:])

        # t's finalize ships on the scalar HWDGE ring (parallel to sync)
        finalize(1, proj_t, e_t, ssq_t, wt_t, wb_t, et_d, st_d,
                 nc.scalar.dma_start)


def _build_nc():
    nc = bacc.Bacc("TRN2", target_bir_lowering=False, debug=False)
    o_d = nc.dram_tensor("o", [MAPS, H, W], F32, kind="ExternalInput").ap()
    t_d = nc.dram_tensor("t", [MAPS, H, W], F32, kind="ExternalInput").ap()
    id_d = nc.dram_tensor("ident", [128, 128], F16, kind="ExternalInput").ap()
    on_d = nc.dram_tensor("ones", [128, 1], F16, kind="ExternalInput").ap()
    ek_d = nc.dram_tensor("ek", [128, 16], F16, kind="ExternalInput").ap()
    eo_d = nc.dram_tensor("eo", [128, 2 * W], F16, kind="ExternalOutput").ap()
    so_d = nc.dram_tensor("so", [128, 4], F32, kind="ExternalOutput").ap()
    et_d = nc.dram_tensor("et", [128, 2 * W], F16, kind="ExternalOutput").ap()
    st_d = nc.dram_tensor("st", [128, 4], F32, kind="ExternalOutput").ap()
    eth_d = nc.dram_tensor("eth", [128, 16], F16, kind="ExternalOutput").ap()
    sth_d = nc.dram_tensor("sth", [1, 16], F32, kind="ExternalOutput").ap()
    etw_d = nc.dram_tensor("etw", [4, 512], F16, kind="ExternalOutput").ap()
    stw_d = nc.dram_tensor("stw", [4, 4], F32, kind="ExternalOutput").ap()
    with tile.TileContext(nc) as tc:
        _body(tc, o_d, t_d, id_d, on_d, ek_d, eo_d, so_d, et_d, st_d,
              eth_d, sth_d, etw_d, stw_d)
    nc.compile()
    return nc


_NC = None


def _get_nc():
    global _NC
    if _NC is None:
        _NC = _build_nc()
    return _NC


_IDENT = np.eye(128, dtype=np.float16)
_ONES = np.ones((128, 1), np.float16)
_EK = np.zeros((128, 16), np.float16)
for _k in range(4):
    _EK[:, 4 * _k + _k] = 1.0
_EK = np.ascontiguousarray(_EK)


def _make_in_maps(output, target):
    in_maps = []
    for i in range(NCORES):
        o = np.ascontiguousarray(output[i * NLOC:(i + 1) * NLOC]).reshape(MAPS, H, W)
        t = np.ascontiguousarray(target[i * NLOC:(i + 1) * NLOC]).reshape(MAPS, H, W)
        in_maps.append({"o": o, "t": t, "ident": _IDENT, "ones": _ONES, "ek": _EK})
    return in_maps


def _core_q(r, ti):
    """Rebuild normalized q [136, 2, 128] (f64) for tensor ti of one core."""
    e_key, s_key = ("eo", "so") if ti == 0 else ("et", "st")
    q = np.empty((MAPS, 2, 128), np.float64)
    e = r[e_key].astype(np.float64).reshape(128, 2, 128)
    sp = r[s_key].astype(np.float64)           # [128, 4] partials
    ssq = np.stack([sp[:, 0] + sp[:, 1], sp[:, 2]], axis=1)  # [128, 2]
    q[0:MAIN] = e / np.sqrt(ssq)[:, :, None]
    eth = r["eth"].astype(np.float64)   # [128 (h), 16 (ti*8+m)]
    sth = r["sth"].astype(np.float64)   # [1, 16]
    etw = r["etw"].astype(np.float64)   # [4, 512]; row k=ti*2+j//4
    stw = r["stw"].astype(np.float64)   # [4, 4]
    for j in range(TAIL):
        col = ti * TAIL + j
        q[MAIN + j, 0] = eth[:, col] / np.sqrt(sth[0, col])
        k, m4 = ti * 2 + j // 4, j % 4
        q[MAIN + j, 1] = etw[k, m4 * 128:(m4 + 1) * 128] / np.sqrt(stw[k, m4])
    return q


def _finish(results):
    A = 0.0
    B = 0.0
    for r in results:
        qo = _core_q(r, 0)
        qt = _core_q(r, 1)
        A += float(np.sum(qo * qt))
        U = qo.reshape(NLOC, C, 2, 128).sum(axis=1)
        V = qt.reshape(NLOC, C, 2, 128).sum(axis=1)
        B += float(np.sum(U * V))
    # sim_pos = 0.5*A/(N*C); sim = 0.5*B/N; loss = -log(sim_pos/sim)/(C*N)
    loss = -np.log(A / (C * B)) / (C * N)
    return np.float32(loss)


def kernel(output, target):
    output = np.asarray(output, dtype=np.float32)
    target = np.asarray(target, dtype=np.float32)
    nc = _get_nc()
    res = run_bass_kernel_spmd(nc, _make_in_maps(output, target), list(range(NCORES)))
    return _finish(res.results)


def profile(output, target):
    """Run once with NTFF tracing; returns max per-core HW exec time in ns."""
    output = np.asarray(output, dtype=np.float32)
    target = np.asarray(target, dtype=np.float32)
    nc = _get_nc()
    res = run_bass_kernel_spmd(
        nc, _make_in_maps(output, target), list(range(NCORES)), trace=True
    )
    return res.exec_time_ns
